# revision 1
# baseline (speedup 1.0000x reference)
"""Concordance-index loss on Trainium2 (8 NeuronCores, raw Bass) — v6.

Staircase decomposition on top of v5's flipped layout (i on partitions, j on
the free dim; DVE compares, PE ones-matmul reduction into PSUM):

Sort i by tm and j by u on host.  comp[:, j] = (u_j > tm_i) is then a
prefix (of length p_j) of the sorted-i order, so for each j-column and each
i-tile the tile is all-zeros (PE skips it), all-ones (conc contribution =
sum_i rgt[i, j]: PE reduces rgt directly, no mask computed), or the ONE
boundary tile, where comp and comp&rgt are computed elementwise over a
narrow contiguous j-window.  The DVE computes the full rgt compare (4x)
plus the small boundary window; the PE accumulates into PSUM.  `total` is
exact host rank math (sum_j p_j) and `tied` is host-enumerated as in v2 --
`conc`, the only genuinely 2-D O(N^2) reduction, stays fully on-device.

SPMD needs ONE program for all 8 cores, but the window boundaries are
data-dependent, so the cores take STRIDED shards: core c = (iq, jh) owns
sorted-i positions t*512 + 4*p + iq (16 tiles of 128) and sorted-j
positions 2*k + jh (4096 columns).  Every core's tile t then spans the same
global-i band [t*512, (t+1)*512), making the per-tile windows near
identical across cores; the compiled program uses their union (correct for
every core: outside its own window a column is genuinely all-ones/zeros,
and the elementwise path is always correct).  The program is built (and
cached) per window structure.
"""

from contextlib import ExitStack

import numpy as np

N = 8192
NCORES = 8
P = 128
NIQ = 4                     # i stride (quarters)
NJH = 2                     # j stride (halves)
IBLK = N // NIQ             # 2048 i's per core
IT = IBLK // P              # 16 i partition-tiles per core
IBAND = P * NIQ             # 512: global sorted-i band per tile
JW = N // NJH               # 4096 j's per core (free dim)
MMW = 512                   # max moving free dim per matmul
NB16 = (2 * JW + 4) * 2     # fp16/partition: u|r rows + ones|zero|pad
NB32 = 2 * IT * 4           # f32 payload/partition: tmi|rei scalars
NOUT = 2                    # dummy [P, 2] f32 main out (sums come via out2)

_CACHE = {}


def _mm_slices(lo, hi):
    out = []
    while lo < hi:
        w = min(MMW, hi - lo)
        out.append((lo, lo + w))
        lo += w
    return out


def _build_nc(windows, repeat=1):
    """windows: per tile t, (lo_t, hi_t): columns [0, lo_t) are all-zeros,
    [lo_t, hi_t) boundary (elementwise), [hi_t, JW) all-ones."""
    import concourse.bass as bass
    from concourse import mybir

    dt = mybir.dt
    Alu = mybir.AluOpType

    nc = bass.Bass()
    xin = nc.declare_dram_parameter("xin", [P, NB16 + NB32], dt.uint8,
                                    isOutput=False)
    out = nc.declare_dram_parameter("out", [P, NOUT], dt.float32,
                                    isOutput=True)
    out2 = nc.declare_dram_parameter("out2", [1, JW], dt.float32,
                                     isOutput=True)

    with (
        nc.sbuf_tensor([P, NB16 + NB32], dt.uint8) as xin_s,
        nc.sbuf_tensor([P, NOUT], dt.float32) as out_s,
        nc.sbuf_tensor([P, JW], dt.float16) as rgt0,
        nc.sbuf_tensor([P, JW], dt.float16) as rgt1,
        nc.sbuf_tensor([P, JW], dt.float16) as comp,
        nc.sbuf_tensor([P, JW], dt.float16) as prod0,
        nc.sbuf_tensor([P, JW], dt.float16) as prod1,
        nc.sbuf_tensor([1, JW], dt.float32) as out2_s,
        nc.psum_tensor([P, JW], dt.float32) as psumT,
        nc.semaphore() as dsem,
        nc.semaphore() as csemP,
        nc.semaphore() as psem,
        nc.semaphore() as vsem,
        nc.Block() as block,
    ):
        xf16_s = xin_s[:, 0:NB16].bitcast(dt.float16)
        xf32_s = xin_s[:, NB16:NB16 + NB32].bitcast(dt.float32)
        uj_row = xf16_s[:, 0:JW]
        rj_row = xf16_s[:, JW:2 * JW]
        ones_w = xf16_s[:, 2 * JW:2 * JW + 1]       # [128, 1] of 1.0
        zero_w = xf16_s[:, 2 * JW + 1:2 * JW + 2]   # [128, 1] of 0.0
        tmi_s = xf32_s[:, 0 * IT:1 * IT]
        rei_s = xf32_s[:, 1 * IT:2 * IT]
        rgts = [rgt0, rgt1]
        prods = [prod0, prod1]

        @block.gpsimd
        def _(g):
            g.dma_start(xin_s[:], xin[:]).then_inc(dsem, 16)
            g.wait_ge(vsem, 1)
            g.dma_start(out[:], out_s[:]).then_inc(dsem, 16)
            g.dma_start(out2[:], out2_s[:]).then_inc(dsem, 16)

        @block.vector
        def _(v):
            v.wait_ge(dsem, 16)

            def one_pass():
                for t in range(IT):
                    col = slice(t, t + 1)
                    lo, hi = windows[t]
                    if lo >= JW:
                        # dead tile: every column's prefix ends below it --
                        # no compute, keep the handshake count
                        v.sem_inc(csemP, 1)
                        continue
                    if t >= 2:
                        v.wait_ge(psem, t)       # PE freed this slot pair
                    # rgt = (r_j < re_i)  [4x] -- only columns >= lo are
                    # ever read (boundary window + all-ones region)
                    v.tensor_scalar(
                        rgts[t % 2][:, lo:JW], rj_row[:, lo:JW],
                        rei_s[:, col], None, Alu.is_lt)
                    if hi > lo:
                        # boundary window: elementwise comp and AND
                        v.tensor_scalar(
                            comp[:, lo:hi], uj_row[:, lo:hi], tmi_s[:, col],
                            None, Alu.is_gt)
                        v.tensor_tensor(
                            prods[t % 2][:, lo:hi], comp[:, lo:hi],
                            rgts[t % 2][:, lo:hi], Alu.min)
                    # drain-then-inc: make the tile's SBUF writes visible
                    # before the PE wakes on csemP
                    v.drain()
                    v.sem_inc(csemP, 1)

            if repeat == 1:
                one_pass()
            else:
                with v.Fori(0, repeat) as _i:
                    one_pass()
            # evacuate PE's conc partials (DMA cannot read PSUM)
            v.wait_ge(psem, (IT + 2) * repeat)
            v.memset(out_s[:], 0.0)
            v.tensor_copy(out2_s[:], psumT[0:1, :])
            v.drain()
            v.sem_inc(vsem, 1)

        @block.tensor
        def _(te):
            te.wait_ge(dsem, 16)

            def one_pass():
                # zero all psum cols (zero weights x finite u-row data)
                last = None
                for mlo, mhi in _mm_slices(0, JW):
                    last = te.matmul(
                        psumT[0:1, mlo:mhi], zero_w, uj_row[:, mlo:mhi],
                        start=True, stop=False, skip_group_check=True)
                last.then_inc(psem, 1)
                for t in range(IT):
                    te.wait_ge(csemP, t + 1)
                    lo, hi = windows[t]
                    last = None
                    # boundary: conc += sum_i comp & rgt
                    for mlo, mhi in _mm_slices(lo, hi):
                        last = te.matmul(
                            psumT[0:1, mlo:mhi], ones_w,
                            prods[t % 2][:, mlo:mhi],
                            start=False, stop=False, skip_group_check=True)
                    # all-ones region: conc += sum_i rgt  (j >= hi)
                    for mlo, mhi in _mm_slices(hi, JW):
                        last = te.matmul(
                            psumT[0:1, mlo:mhi], ones_w,
                            rgts[t % 2][:, mlo:mhi],
                            start=False, stop=False, skip_group_check=True)
                    if last is None:
                        te.nop().then_inc(psem, 1)
                    else:
                        last.then_inc(psem, 1)
                # flush the systolic array: MMs retire before their columns
                # finish draining into PSUM, and the DVE evacuation copy
                # must not race the in-flight writes of the final matmuls
                te.drain()
                te.nop().then_inc(psem, 1)

            if repeat == 1:
                one_pass()
            else:
                with te.Fori(0, repeat) as _i:
                    one_pass()

    return nc


def _encode(event_indicator, event_time, estimate):
    d = np.asarray(event_indicator).reshape(-1).astype(bool)
    t = np.asarray(event_time, dtype=np.float32).reshape(-1)
    r = np.asarray(estimate, dtype=np.float32).reshape(-1)
    assert t.shape[0] == N

    tv = np.unique(t)
    trk = np.searchsorted(tv, t).astype(np.float32)
    # fp16 must represent trk and trk+0.5 exactly -> need trk+1 < 1024
    assert len(tv) + 2 < 1024, "t ranks must stay fp16-exact incl. +0.5"
    return d, t, r, trk


def _structure(event_indicator, event_time, estimate):
    """Sorted orders, encodings, exact total, and per-tile union windows."""
    d, _t, r, trk = _encode(event_indicator, event_time, estimate)

    u = (trk + np.float32(0.5) * (~d).astype(np.float32)).astype(np.float16)
    tm = np.where(d, trk, np.float32(32768.0)).astype(np.float16)

    rv = np.unique(r)
    m = len(rv)
    assert m + 1024 < 31744, "r rank embedding must stay in normal fp16 range"
    emb = (np.arange(m, dtype=np.uint16) + np.uint16(1024)).view(np.float16)
    r_e = emb[np.searchsorted(rv, r)]

    iord = np.argsort(tm.astype(np.float32), kind="stable")  # i by tm asc
    jord = np.argsort(u.astype(np.float32), kind="stable")   # j by u asc
    tms = tm[iord].astype(np.float32)
    us = u[jord].astype(np.float32)

    # p_j = #{i: tm_i < u_j}: prefix length in sorted-i order (exact ints)
    pj_sorted = np.searchsorted(tms, us, side="left")
    total = float(pj_sorted.sum())

    # union windows over cores: core (iq, jh) takes j positions 2k+jh; its
    # tile t spans global-i band [t*512, (t+1)*512).  Column k is all-zeros
    # for tile t iff pj <= t*512, all-ones iff pj >= (t+1)*512.
    windows = []
    for t in range(IT):
        lo_u, hi_u = JW, 0
        for jh in range(NJH):
            pj_loc = pj_sorted[jh::NJH]
            lo = int(np.searchsorted(pj_loc, t * IBAND, side="right"))
            hi = int(np.searchsorted(pj_loc, (t + 1) * IBAND, side="left"))
            lo_u, hi_u = min(lo_u, lo), max(hi_u, hi)
        lo_u &= ~1   # keep 4B alignment for the fp16 window ops
        windows.append((lo_u, hi_u))
    return d, r, trk, u, tm, r_e, iord, jord, total, tuple(windows)


def _prep_inputs(event_indicator, event_time, estimate):
    (_d, _r, _trk, u, tm, r_e, iord, jord, _total,
     _windows) = _structure(event_indicator, event_time, estimate)

    in_maps = []
    for c in range(NCORES):
        iq, jh = divmod(c, NJH)
        li = np.arange(IBLK)
        isel = iord[(li // P) * IBAND + (li % P) * NIQ + iq]
        jsel = jord[jh::NJH]
        tmi = np.ascontiguousarray(
            tm[isel].astype(np.float32).reshape(IT, P).T)
        rei = np.ascontiguousarray(
            r_e[isel].astype(np.float32).reshape(IT, P).T)
        b32 = np.ascontiguousarray(
            np.concatenate([tmi, rei], axis=1)).view(np.uint8).reshape(P, -1)
        row16 = np.concatenate([
            u[jsel], r_e[jsel],
            np.array([1.0, 0.0, 0.0, 0.0], dtype=np.float16),
        ])
        b16 = np.ascontiguousarray(
            np.broadcast_to(row16[None, :], (P, 2 * JW + 4))).view(np.uint8)
        in_maps.append({
            "xin": np.ascontiguousarray(np.concatenate([b16, b32], axis=1)),
        })
    return in_maps


def _tied_host(event_indicator, event_time, estimate):
    """Exact tied_risk count (see kernel_v2 docstring)."""
    d, _t, r, trk = _encode(event_indicator, event_time, estimate)

    thr = np.float32(1e-8)
    order = np.argsort(r, kind="stable")
    rs = r[order]
    lo = np.zeros(N, dtype=np.int64)
    hi = np.zeros(N, dtype=np.int64)
    p = 0
    for k in range(N):
        while np.abs(rs[k] - rs[p]) > thr:
            p += 1
        lo[k] = p
    p = N - 1
    for k in range(N - 1, -1, -1):
        while np.abs(rs[k] - rs[p]) > thr:
            p -= 1
        hi[k] = p

    cnt = hi - lo + 1
    T = int(cnt.sum())
    K = np.repeat(np.arange(N, dtype=np.int64), cnt)
    offs = np.concatenate(([0], np.cumsum(cnt)[:-1]))
    Ppos = np.arange(T, dtype=np.int64) - np.repeat(offs, cnt) + np.repeat(lo, cnt)
    i_idx = order[K]
    j_idx = order[Ppos]
    comp = d[i_idx] & (
        (trk[i_idx] < trk[j_idx])
        | ((trk[i_idx] == trk[j_idx]) & (~d[j_idx]))
    )
    return float(comp.sum())


def _finish(results, total, tied):
    conc = np.float64(0.0)
    for res in results:
        conc += res["out2"].astype(np.float64).sum()
    disc = total - conc - tied
    loss = (disc + 0.5 * tied) / (disc + conc + tied + 1e-7)
    return np.asarray(1.0 - loss, dtype=np.float32)


def kernel(event_indicator, event_time, estimate):
    from concourse.bass_utils import run_bass_kernel_spmd

    st = _structure(event_indicator, event_time, estimate)
    total, windows = st[8], st[9]
    in_maps = _prep_inputs(event_indicator, event_time, estimate)
    tied = _tied_host(event_indicator, event_time, estimate)

    if _CACHE.get("windows") != windows:
        _CACHE["nc"] = _build_nc(windows)
        _CACHE["windows"] = windows
        _CACHE["primed"] = False
    nc = _CACHE["nc"]
    # Priming run: on the literal first execution after device load, the
    # PSUM zero-pass write of the final matmul slice does not take effect
    # (boot-state PSUM garbage survives under the accumulation for the last
    # ~46 columns); every execution >= 2 is exact.  Execute twice and use
    # the steady-state result.
    if not _CACHE.get("primed"):
        run_bass_kernel_spmd(nc, in_maps, core_ids=list(range(NCORES)))
        _CACHE["primed"] = True
    out = run_bass_kernel_spmd(nc, in_maps, core_ids=list(range(NCORES)))
    return _finish(out.results, total, tied)

